# revision 15
# baseline (speedup 1.0000x reference)
"""Trainium2 Bass kernel for ConstrainedSparseClusterDecomposition.

For x (16, 8192, 128) f32 and dictionary (64, 128) f32 (orthonormal rows):
  scores   = x @ dictionary.T ; q = softmax(scores/6)
  top-4 hard routing via masked softmax (gather-free dense weights)
  x_common = wk @ dictionary ; x_resid = x - x_common
  aux_loss = 0.5*KL(p||q) + 0.1*ortho(dictionary), p built from the
  global per-cluster sums f_c = sum_tokens q (all-reduced on device).

Sharding: data-parallel over flattened tokens across 8 NeuronCores
(16384 tokens/core); dictionary replicated; f all-reduced on device;
scalar KL partials summed on host during the gather.

Per-core structure (three sweeps, collective hidden under sweep B):
  sweep A: load x -> scores -> q -> f partials + top-4 dense weights
  collective: AllReduce f (TOPSP/SDMA, overlaps sweep B)
  sweep B: transpose weights -> x_common/x_resid -> DMA out
  sweep C (interleaved with B): KL terms from stored q
"""

import numpy as np
import ml_dtypes

import concourse.bacc as bacc
import concourse.mybir as mybir
from concourse.tile import TileContext
from concourse.masks import make_identity
from concourse import bass_utils

F32 = mybir.dt.float32
BF16 = mybir.dt.bfloat16
FP16 = mybir.dt.float16
AX = mybir.AxisListType
OP = mybir.AluOpType
AF = mybir.ActivationFunctionType

B, N, D, C, K = 16, 8192, 128, 64, 4
NCORES = 8
M_TOTAL = B * N
TOK = M_TOTAL // NCORES              # 16384 tokens per core
GRP = 128
ST = 8                               # groups per supertile
NST = TOK // (GRP * ST)              # 16 supertiles
NGRP = TOK // GRP                    # 128 groups
TEMP = 6.0
INV_TEMP = 1.0 / TEMP

# tuning flags
KL_DT = FP16      # dtype for the KL elementwise chain (reductions stay f32)
F_BF16 = True     # stream q to the f-matmul in bf16 (4x faster on PE)
EXP_ACCUM = False  # per-group exp+accum regressed: extra ACC_READ op per group
C_OFFSET = NST    # C fully after B (interleaving throttled B's PSUM recycling)

_CACHED = {}


def _build():
    nc = bacc.Bacc("TRN2", target_bir_lowering=False, debug=False,
                   num_devices=NCORES)

    x_d = nc.dram_tensor("x", [TOK, D], F32, kind="ExternalInput")
    dictT_d = nc.dram_tensor("dictT", [D, C], F32, kind="ExternalInput")
    dictbf_d = nc.dram_tensor("dictbf", [C, D], BF16, kind="ExternalInput")
    xc_d = nc.dram_tensor("xc", [TOK, D], F32, kind="ExternalOutput")
    xr_d = nc.dram_tensor("xr", [TOK, D], F32, kind="ExternalOutput")
    kl_d = nc.dram_tensor("kl", [1, 1], F32, kind="ExternalOutput")
    ortho_d = nc.dram_tensor("ortho", [1, 1], F32, kind="ExternalOutput")

    f_loc_d = nc.dram_tensor("f_loc", [C], F32)
    f_red_d = nc.dram_tensor("f_red", [NCORES * C], F32, addr_space="Shared")

    with TileContext(nc) as tc:
        with (
            tc.tile_pool(name="consts", bufs=1) as consts,
            tc.tile_pool(name="big", bufs=1) as big,
            tc.tile_pool(name="work", bufs=2) as work,
            tc.tile_pool(name="stats", bufs=2) as stats,
            tc.tile_pool(name="persist", bufs=1) as persist,
        ):
            ident = consts.tile([128, 128], F32)
            make_identity(nc, ident)
            identb = consts.tile([128, 128], BF16)
            make_identity(nc, identb)
            ones_col = consts.tile([128, 1], F32)
            nc.vector.memset(ones_col, 1.0)
            ones_col_bf = consts.tile([128, 1], BF16)
            nc.vector.memset(ones_col_bf, 1.0)
            ones_r1 = consts.tile([1, 128], F32)
            nc.vector.memset(ones_r1, 1.0)
            dictT_sb = consts.tile([D, C], F32)
            nc.sync.dma_start(dictT_sb, dictT_d.ap())
            dictbf_sb = consts.tile([C, D], BF16)
            nc.sync.dma_start(dictbf_sb, dictbf_d.ap())

            xs_arr = big.tile([128, TOK], F32)        # 64 KiB/partition
            q_arr = big.tile([128, NGRP * C], F32)    # 32 KiB/partition
            wd_arr = big.tile([128, NGRP * C], BF16)  # 16 KiB/partition
            A_arr = persist.tile([128, NGRP], F32)
            L_arr = persist.tile([128, NGRP], F32)

            # ---------------- ortho loss (tiny, independent) -------------
            with tc.tile_pool(name="ps_o", bufs=1, space="PSUM") as pso:
                gram_p = pso.tile([C, C], F32)
                nc.tensor.matmul(gram_p, dictT_sb, dictT_sb, start=True,
                                 stop=True)
                delta = consts.tile([C, C], F32)
                nc.vector.tensor_sub(delta, gram_p, ident[:C, :C])
                dsq = consts.tile([C, C], F32)
                nc.vector.tensor_mul(dsq, delta, delta)
                ocol = consts.tile([C, 1], F32)
                nc.vector.tensor_reduce(ocol, dsq, AX.X, OP.add)
                o_p = pso.tile([1, 1], F32)
                nc.tensor.matmul(o_p, ocol, ones_col[:C, :], start=True,
                                 stop=True)
                o_sb = consts.tile([1, 1], F32)
                nc.scalar.mul(o_sb, o_p, 1.0 / (C * C))
                nc.sync.dma_start(ortho_d.ap(), o_sb)

            # -------- sweep A: scores -> q -> f + top-4 weights ----------
            with (
                tc.tile_pool(name="ps_xt", bufs=2, space="PSUM") as ps_xt,
                tc.tile_pool(name="ps_sc", bufs=2, space="PSUM") as ps_sc,
                tc.tile_pool(name="ps_f", bufs=1, space="PSUM") as ps_f,
            ):
                f_p = ps_f.tile([1, ST * C], F32)

                for st in range(NST):
                    row0 = st * ST * GRP
                    xs = xs_arr[:, row0:row0 + ST * GRP]
                    src = x_d.ap()[row0:row0 + ST * GRP, :].rearrange(
                        "(g p) d -> p g d", p=128)
                    nc.sync.dma_start(
                        xs.rearrange("p (g d) -> p g d", d=128), src)

                    xT_sb = work.tile([128, ST * 128], F32, tag="xT")
                    for h in range(2):
                        xT_p = ps_xt.tile([128, 512], F32, tag="xT_p")
                        for j in range(4):
                            g = h * 4 + j
                            nc.tensor.transpose(
                                xT_p[:, j * 128:(j + 1) * 128],
                                xs[:, g * 128:(g + 1) * 128], ident)
                        nc.scalar.copy(xT_sb[:, h * 512:(h + 1) * 512], xT_p)

                    sc_p = ps_sc.tile([128, ST * C], F32, tag="sc")
                    for g in range(ST):
                        nc.tensor.matmul(
                            sc_p[:, g * C:(g + 1) * C],
                            xT_sb[:, g * 128:(g + 1) * 128],
                            dictT_sb, start=True, stop=True)

                    e_sb = work.tile([128, ST * C], F32, tag="e")
                    e3 = e_sb.rearrange("p (g c) -> p g c", c=C)
                    S_t = stats.tile([128, ST], F32, tag="S")
                    if EXP_ACCUM:
                        for g in range(ST):
                            nc.scalar.activation(
                                e_sb[:, g * C:(g + 1) * C],
                                sc_p[:, g * C:(g + 1) * C],
                                AF.Exp, scale=INV_TEMP,
                                accum_out=S_t[:, g:g + 1])
                    else:
                        nc.scalar.activation(e_sb, sc_p, AF.Exp,
                                             scale=INV_TEMP)
                        nc.vector.tensor_reduce(S_t, e3, AX.X, OP.add)
                    rS_t = stats.tile([128, ST], F32, tag="rS")
                    nc.vector.reciprocal(rS_t, S_t)
                    q_sl = q_arr[:, st * ST * C:(st + 1) * ST * C]
                    q3 = q_sl.rearrange("p (g c) -> p g c", c=C)
                    nc.vector.tensor_tensor(
                        q3, e3, rS_t.to_broadcast([128, ST, C]), OP.mult)

                    # f partial: ones^T @ q -> (1, ST*C), PSUM-accumulated
                    if F_BF16:
                        qbf = work.tile([128, ST * C], BF16, tag="qbf")
                        nc.vector.tensor_copy(qbf, q_sl)
                        nc.tensor.matmul(f_p, ones_col_bf, qbf,
                                         start=(st == 0),
                                         stop=(st == NST - 1))
                    else:
                        nc.tensor.matmul(f_p, ones_col, q_sl,
                                         start=(st == 0),
                                         stop=(st == NST - 1))

                    # top-4 mask and normalized dense weights (bf16)
                    m8 = stats.tile([128, ST * 8], F32, tag="m8")
                    for g in range(ST):
                        nc.vector.max(m8[:, g * 8:(g + 1) * 8],
                                      q_sl[:, g * C:(g + 1) * C])
                    m83 = m8.rearrange("p (g e) -> p g e", e=8)
                    q4s = stats.tile([128, ST], F32, tag="q4s")
                    nc.vector.tensor_reduce(q4s, m83[:, :, 0:4], AX.X, OP.add)
                    rq4 = stats.tile([128, ST], F32, tag="rq4")
                    nc.vector.reciprocal(rq4, q4s)
                    thr = stats.tile([128, ST], F32, tag="thr")
                    nc.vector.tensor_copy(thr, m83[:, :, 3:4])

                    mask = work.tile([128, ST * C], F32, tag="mask")
                    mask3 = mask.rearrange("p (g c) -> p g c", c=C)
                    nc.vector.tensor_tensor(
                        mask3, q3, thr.to_broadcast([128, ST, C]), OP.is_ge)
                    wdu = work.tile([128, ST * C], F32, tag="wdu")
                    nc.vector.tensor_tensor(wdu, q_sl, mask, OP.mult)
                    wd_sl = wd_arr[:, st * ST * C:(st + 1) * ST * C]
                    nc.vector.tensor_tensor(
                        wd_sl.rearrange("p (g c) -> p g c", c=C),
                        wdu.rearrange("p (g c) -> p g c", c=C),
                        rq4.to_broadcast([128, ST, C]), OP.mult)

                # fold the 8 per-group columns: f_c = sum_g f_p[0, 64g+c]
                f_row = persist.tile([1, C], F32)
                fp_view = f_p.rearrange("o (g c) -> o c g", c=C)
                nc.vector.tensor_reduce(f_row, fp_view, AX.X, OP.add)
                nc.sync.dma_start(
                    f_loc_d.ap().rearrange("(o c) -> o c", o=1), f_row)

            # ------ all-gather f across cores (overlaps sweep B) ---------
            nc.gpsimd.collective_compute(
                "AllGather", OP.bypass,
                replica_groups=[list(range(NCORES))],
                ins=[f_loc_d.ap()], outs=[f_red_d.ap()])

            with tc.tile_pool(name="ps_bc", bufs=1, space="PSUM") as ps_bc:
                fg_row = persist.tile([1, NCORES * C], F32)
                nc.sync.dma_start(
                    fg_row, f_red_d.ap().rearrange("(o c) -> o c", o=1))
                fr_row = persist.tile([1, C], F32)
                nc.vector.tensor_reduce(
                    fr_row, fg_row.rearrange("o (r c) -> o c r", c=C),
                    AX.X, OP.add)
                g_row8 = persist.tile([1, ST * C], F32)
                g_row = g_row8[:, 0:C]
                nc.vector.reciprocal(g_row, fr_row)
                if KL_DT == FP16:
                    # keep v = q*g and a = q*v inside fp16's normal range;
                    # kl_tok = L/A - ln A is exactly invariant to this scale
                    nc.vector.tensor_scalar_mul(g_row, g_row, 4096.0)
                for g in range(1, ST):
                    nc.vector.tensor_copy(g_row8[:, g * C:(g + 1) * C],
                                          g_row)
                grep_p = ps_bc.tile([128, ST * C], F32)
                nc.tensor.matmul(grep_p, ones_r1, g_row8, start=True,
                                 stop=True)

                # ---------- sweep B (outputs), then sweep C (KL) ---------
                # All of B is emitted before any C op so the per-engine
                # static order lets B proceed while the collective that
                # gates C is still in flight.
                with (
                    tc.tile_pool(name="ps_wt", bufs=2, space="PSUM") as ps_wt,
                    tc.tile_pool(name="ps_xc", bufs=2, space="PSUM") as ps_xc,
                ):
                    def emit_B(st):
                        row0 = st * ST * GRP
                        xs = xs_arr[:, row0:row0 + ST * GRP]
                        wd_sl = wd_arr[:, st * ST * C:(st + 1) * ST * C]

                        wT_p = ps_wt.tile([C, ST * 128], BF16, tag="wT_p")
                        for g in range(ST):
                            nc.tensor.transpose(
                                wT_p[:, g * 128:(g + 1) * 128],
                                wd_sl[:, g * C:(g + 1) * C], identb)
                        wT_sb = work.tile([C, ST * 128], BF16, tag="wT")
                        nc.scalar.copy(wT_sb, wT_p)

                        for h in range(2):
                            xc_p = ps_xc.tile([128, 512], F32, tag="xc_p")
                            for j in range(4):
                                g = h * 4 + j
                                nc.tensor.matmul(
                                    xc_p[:, j * 128:(j + 1) * 128],
                                    wT_sb[:, g * 128:(g + 1) * 128],
                                    dictbf_sb, start=True, stop=True)
                            res_sb = work.tile([128, 512], F32, tag="res")
                            nc.vector.tensor_sub(
                                res_sb, xs[:, h * 512:(h + 1) * 512], xc_p)
                            xc_sb = work.tile([128, 512], F32, tag="xc")
                            nc.scalar.copy(xc_sb, xc_p)
                            lo = row0 + h * 4 * GRP
                            nc.sync.dma_start(
                                xc_d.ap()[lo:lo + 512, :].rearrange(
                                    "(g p) d -> p g d", p=128),
                                xc_sb.rearrange("p (g d) -> p g d", d=128))
                            nc.sync.dma_start(
                                xr_d.ap()[lo:lo + 512, :].rearrange(
                                    "(g p) d -> p g d", p=128),
                                res_sb.rearrange("p (g d) -> p g d", d=128))

                    def emit_C(st):
                        q_sl = q_arr[:, st * ST * C:(st + 1) * ST * C]
                        v_sb = work.tile([128, ST * C], KL_DT, tag="v")
                        nc.vector.tensor_tensor(v_sb, q_sl, grep_p, OP.mult)
                        a_sb = work.tile([128, ST * C], KL_DT, tag="a")
                        nc.vector.tensor_tensor(a_sb, q_sl, v_sb, OP.mult)
                        u_sb = work.tile([128, ST * C], KL_DT, tag="u")
                        nc.scalar.activation(u_sb, v_sb, AF.Ln)
                        t_sb = work.tile([128, ST * C], KL_DT, tag="t")
                        nc.vector.tensor_tensor(t_sb, a_sb, u_sb, OP.mult)
                        nc.vector.tensor_reduce(
                            A_arr[:, st * ST:(st + 1) * ST],
                            a_sb.rearrange("p (g c) -> p g c", c=C),
                            AX.X, OP.add)
                        nc.vector.tensor_reduce(
                            L_arr[:, st * ST:(st + 1) * ST],
                            t_sb.rearrange("p (g c) -> p g c", c=C),
                            AX.X, OP.add)

                    # B first; C offset so the collective completes before
                    # the DVE's static program order reaches C(0)
                    for st in range(NST):
                        emit_B(st)
                        if st >= C_OFFSET:
                            emit_C(st - C_OFFSET)
                    for st in range(NST - C_OFFSET, NST):
                        emit_C(st)

                    # ---- KL tail ----
                    rA = persist.tile([128, NGRP], F32)
                    nc.vector.reciprocal(rA, A_arr)
                    lnA = persist.tile([128, NGRP], F32)
                    nc.scalar.activation(lnA, A_arr, AF.Ln)
                    kt = persist.tile([128, NGRP], F32)
                    nc.vector.tensor_tensor(kt, L_arr, rA, OP.mult)
                    nc.vector.tensor_sub(kt, kt, lnA)
                    klcol = persist.tile([128, 1], F32)
                    nc.vector.tensor_reduce(klcol, kt, AX.X, OP.add)
                    kl_p = ps_bc.tile([1, 1], F32)
                    nc.tensor.matmul(kl_p, klcol, ones_col, start=True,
                                     stop=True)
                    kl_sb = persist.tile([1, 1], F32)
                    nc.scalar.mul(kl_sb, kl_p, 0.5 / M_TOTAL)
                    nc.sync.dma_start(kl_d.ap(), kl_sb)

    nc.compile()
    return nc


def kernel(x: np.ndarray, dictionary: np.ndarray):
    if "nc" not in _CACHED:
        _CACHED["nc"] = _build()
    nc = _CACHED["nc"]

    x_flat = np.ascontiguousarray(x.reshape(M_TOTAL, D).astype(np.float32))
    dT = np.ascontiguousarray(dictionary.T.astype(np.float32))
    dbf = np.ascontiguousarray(dictionary.astype(ml_dtypes.bfloat16))

    in_maps = [{
        "x": x_flat[c * TOK:(c + 1) * TOK],
        "dictT": dT,
        "dictbf": dbf,
    } for c in range(NCORES)]

    res = bass_utils.run_bass_kernel_spmd(nc, in_maps,
                                          core_ids=list(range(NCORES)))
    outs = res.results

    xc = np.concatenate([outs[c]["xc"] for c in range(NCORES)], axis=0)
    xr = np.concatenate([outs[c]["xr"] for c in range(NCORES)], axis=0)
    kl_scaled = np.sum([outs[c]["kl"][0, 0] for c in range(NCORES)],
                       dtype=np.float32)
    ortho = outs[0]["ortho"][0, 0]
    aux = np.float32(kl_scaled + np.float32(0.1) * ortho)

    return (xc.reshape(B, N, D), xr.reshape(B, N, D), aux)


# revision 19
# speedup vs baseline: 1.1490x; 1.1490x over previous
"""Trainium2 Bass kernel for ConstrainedSparseClusterDecomposition.

For x (16, 8192, 128) f32 and dictionary (64, 128) f32 (orthonormal rows):
  scores   = x @ dictionary.T ; q = softmax(scores/6)
  top-4 hard routing via masked softmax (gather-free dense weights)
  x_common = wk @ dictionary ; x_resid = x - x_common
  aux_loss = 0.5*KL(p||q) + 0.1*ortho(dictionary), p built from the
  global per-cluster sums f_c = sum_tokens q (all-reduced on device).

Sharding: data-parallel over flattened tokens across 8 NeuronCores
(16384 tokens/core); dictionary replicated; f all-reduced on device;
scalar KL partials summed on host during the gather.

Per-core structure (three sweeps, collective hidden under sweep B):
  sweep A: load x -> scores -> q -> f partials + top-4 dense weights
  collective: AllReduce f (TOPSP/SDMA, overlaps sweep B)
  sweep B: transpose weights -> x_common/x_resid -> DMA out
  sweep C (interleaved with B): KL terms from stored q
"""

import numpy as np
import ml_dtypes

import concourse.bacc as bacc
import concourse.mybir as mybir
from concourse.tile import TileContext
from concourse.masks import make_identity
from concourse import bass_utils

F32 = mybir.dt.float32
BF16 = mybir.dt.bfloat16
FP16 = mybir.dt.float16
AX = mybir.AxisListType
OP = mybir.AluOpType
AF = mybir.ActivationFunctionType

B, N, D, C, K = 16, 8192, 128, 64, 4
NCORES = 8
M_TOTAL = B * N
TOK = M_TOTAL // NCORES              # 16384 tokens per core
GRP = 128
ST = 8                               # groups per supertile
NST = TOK // (GRP * ST)              # 16 supertiles
NGRP = TOK // GRP                    # 128 groups
TEMP = 6.0
INV_TEMP = 1.0 / TEMP

# tuning flags
KL_DT = FP16      # dtype for the KL elementwise chain (reductions stay f32)
F_BF16 = False    # bf16 f-matmul regressed wall time (extra copy + dep chain)
EXP_ACCUM = False  # per-group exp+accum regressed: extra ACC_READ op per group
C_OFFSET = NST    # C fully after B (interleaving throttled B's PSUM recycling)
F_DELAY = 2       # supertiles of delay before the f-matmul consumes q

_CACHED = {}


def _build():
    nc = bacc.Bacc("TRN2", target_bir_lowering=False, debug=False,
                   num_devices=NCORES)

    x_d = nc.dram_tensor("x", [TOK, D], F32, kind="ExternalInput")
    dictT_d = nc.dram_tensor("dictT", [D, C], F32, kind="ExternalInput")
    dictbf_d = nc.dram_tensor("dictbf", [C, D], BF16, kind="ExternalInput")
    xc_d = nc.dram_tensor("xc", [TOK, D], F32, kind="ExternalOutput")
    xr_d = nc.dram_tensor("xr", [TOK, D], F32, kind="ExternalOutput")
    kl_d = nc.dram_tensor("kl", [1, 1], F32, kind="ExternalOutput")
    ortho_d = nc.dram_tensor("ortho", [1, 1], F32, kind="ExternalOutput")

    f_loc_d = nc.dram_tensor("f_loc", [C], F32)
    f_red_d = nc.dram_tensor("f_red", [NCORES * C], F32, addr_space="Shared")

    with TileContext(nc) as tc:
        with (
            tc.tile_pool(name="consts", bufs=1) as consts,
            tc.tile_pool(name="big", bufs=1) as big,
            tc.tile_pool(name="work", bufs=2) as work,
            tc.tile_pool(name="stats", bufs=2) as stats,
            tc.tile_pool(name="persist", bufs=1) as persist,
        ):
            ident = consts.tile([128, 128], F32)
            make_identity(nc, ident)
            identb = consts.tile([128, 128], BF16)
            make_identity(nc, identb)
            ones_col = consts.tile([128, 1], F32)
            nc.vector.memset(ones_col, 1.0)
            ones_col_bf = consts.tile([128, 1], BF16)
            nc.vector.memset(ones_col_bf, 1.0)
            ones_r1 = consts.tile([1, 128], F32)
            nc.vector.memset(ones_r1, 1.0)
            dictT_sb = consts.tile([D, C], F32)
            nc.sync.dma_start(dictT_sb, dictT_d.ap())
            dictbf_sb = consts.tile([C, D], BF16)
            nc.sync.dma_start(dictbf_sb, dictbf_d.ap())

            xs_arr = big.tile([128, TOK], F32)        # 64 KiB/partition
            q_arr = big.tile([128, NGRP * C], F32)    # 32 KiB/partition
            wd_arr = big.tile([128, NGRP * C], BF16)  # 16 KiB/partition
            A_arr = persist.tile([128, NGRP], F32)
            L_arr = persist.tile([128, NGRP], F32)

            # ---------------- ortho loss (tiny, independent) -------------
            with tc.tile_pool(name="ps_o", bufs=1, space="PSUM") as pso:
                gram_p = pso.tile([C, C], F32)
                nc.tensor.matmul(gram_p, dictT_sb, dictT_sb, start=True,
                                 stop=True)
                delta = consts.tile([C, C], F32)
                nc.vector.tensor_sub(delta, gram_p, ident[:C, :C])
                dsq = consts.tile([C, C], F32)
                nc.vector.tensor_mul(dsq, delta, delta)
                ocol = consts.tile([C, 1], F32)
                nc.vector.tensor_reduce(ocol, dsq, AX.X, OP.add)
                o_p = pso.tile([1, 1], F32)
                nc.tensor.matmul(o_p, ocol, ones_col[:C, :], start=True,
                                 stop=True)
                o_sb = consts.tile([1, 1], F32)
                nc.scalar.mul(o_sb, o_p, 1.0 / (C * C))
                nc.sync.dma_start(ortho_d.ap(), o_sb)

            # -------- sweep A: scores -> q -> f + top-4 weights ----------
            with (
                tc.tile_pool(name="ps_xt", bufs=2, space="PSUM") as ps_xt,
                tc.tile_pool(name="ps_sc", bufs=2, space="PSUM") as ps_sc,
                tc.tile_pool(name="ps_f", bufs=1, space="PSUM") as ps_f,
            ):
                f_p = ps_f.tile([1, ST * C], F32)

                for st in range(NST):
                    row0 = st * ST * GRP
                    xs = xs_arr[:, row0:row0 + ST * GRP]
                    src = x_d.ap()[row0:row0 + ST * GRP, :].rearrange(
                        "(g p) d -> p g d", p=128)
                    nc.sync.dma_start(
                        xs.rearrange("p (g d) -> p g d", d=128), src)

                    xT_sb = work.tile([128, ST * 128], F32, tag="xT")
                    for h in range(2):
                        xT_p = ps_xt.tile([128, 512], F32, tag="xT_p")
                        for j in range(4):
                            g = h * 4 + j
                            nc.tensor.transpose(
                                xT_p[:, j * 128:(j + 1) * 128],
                                xs[:, g * 128:(g + 1) * 128], ident)
                        nc.scalar.copy(xT_sb[:, h * 512:(h + 1) * 512], xT_p)

                    sc_p = ps_sc.tile([128, ST * C], F32, tag="sc")
                    for g in range(ST):
                        nc.tensor.matmul(
                            sc_p[:, g * C:(g + 1) * C],
                            xT_sb[:, g * 128:(g + 1) * 128],
                            dictT_sb, start=True, stop=True)

                    e_sb = work.tile([128, ST * C], F32, tag="e")
                    e3 = e_sb.rearrange("p (g c) -> p g c", c=C)
                    S_t = stats.tile([128, ST], F32, tag="S")
                    if EXP_ACCUM:
                        for g in range(ST):
                            nc.scalar.activation(
                                e_sb[:, g * C:(g + 1) * C],
                                sc_p[:, g * C:(g + 1) * C],
                                AF.Exp, scale=INV_TEMP,
                                accum_out=S_t[:, g:g + 1])
                    else:
                        nc.scalar.activation(e_sb, sc_p, AF.Exp,
                                             scale=INV_TEMP)
                        nc.vector.tensor_reduce(S_t, e3, AX.X, OP.add)
                    rS_t = stats.tile([128, ST], F32, tag="rS")
                    nc.vector.reciprocal(rS_t, S_t)
                    q_sl = q_arr[:, st * ST * C:(st + 1) * ST * C]
                    q3 = q_sl.rearrange("p (g c) -> p g c", c=C)
                    nc.vector.tensor_tensor(
                        q3, e3, rS_t.to_broadcast([128, ST, C]), OP.mult)

                    # f partial: ones^T @ q -> (1, ST*C), PSUM-accumulated.
                    # Emitted with a 2-supertile delay so the PE never waits
                    # on the DVE-produced q mid-sweep (keeps HAM warm).
                    if st >= F_DELAY:
                        fs = st - F_DELAY
                        nc.tensor.matmul(
                            f_p, ones_col,
                            q_arr[:, fs * ST * C:(fs + 1) * ST * C],
                            start=(fs == 0), stop=(fs == NST - 1))

                    # top-4 mask and normalized dense weights (bf16)
                    m8 = stats.tile([128, ST * 8], F32, tag="m8")
                    for g in range(ST):
                        nc.vector.max(m8[:, g * 8:(g + 1) * 8],
                                      q_sl[:, g * C:(g + 1) * C])
                    m83 = m8.rearrange("p (g e) -> p g e", e=8)
                    q4s = stats.tile([128, ST], F32, tag="q4s")
                    nc.vector.tensor_reduce(q4s, m83[:, :, 0:4], AX.X, OP.add)
                    rq4 = stats.tile([128, ST], F32, tag="rq4")
                    nc.vector.reciprocal(rq4, q4s)
                    thr = stats.tile([128, ST], F32, tag="thr")
                    nc.vector.tensor_copy(thr, m83[:, :, 3:4])

                    mask = work.tile([128, ST * C], F32, tag="mask")
                    mask3 = mask.rearrange("p (g c) -> p g c", c=C)
                    nc.vector.tensor_tensor(
                        mask3, q3, thr.to_broadcast([128, ST, C]), OP.is_ge)
                    wdu = work.tile([128, ST * C], F32, tag="wdu")
                    nc.vector.tensor_tensor(wdu, q_sl, mask, OP.mult)
                    wd_sl = wd_arr[:, st * ST * C:(st + 1) * ST * C]
                    nc.vector.tensor_tensor(
                        wd_sl.rearrange("p (g c) -> p g c", c=C),
                        wdu.rearrange("p (g c) -> p g c", c=C),
                        rq4.to_broadcast([128, ST, C]), OP.mult)

                # delayed tail of the f accumulation
                for fs in range(NST - F_DELAY, NST):
                    nc.tensor.matmul(
                        f_p, ones_col,
                        q_arr[:, fs * ST * C:(fs + 1) * ST * C],
                        start=(fs == 0), stop=(fs == NST - 1))

                # fold the 8 per-group columns: f_c = sum_g f_p[0, 64g+c]
                f_row = persist.tile([1, C], F32)
                fp_view = f_p.rearrange("o (g c) -> o c g", c=C)
                nc.vector.tensor_reduce(f_row, fp_view, AX.X, OP.add)
                nc.sync.dma_start(
                    f_loc_d.ap().rearrange("(o c) -> o c", o=1), f_row)

            # ------ all-gather f across cores (overlaps sweep B) ---------
            nc.gpsimd.collective_compute(
                "AllGather", OP.bypass,
                replica_groups=[list(range(NCORES))],
                ins=[f_loc_d.ap()], outs=[f_red_d.ap()])

            with tc.tile_pool(name="ps_bc", bufs=1, space="PSUM") as ps_bc:
                fg_row = persist.tile([1, NCORES * C], F32)
                nc.sync.dma_start(
                    fg_row, f_red_d.ap().rearrange("(o c) -> o c", o=1))
                fr_row = persist.tile([1, C], F32)
                nc.vector.tensor_reduce(
                    fr_row, fg_row.rearrange("o (r c) -> o c r", c=C),
                    AX.X, OP.add)
                g_row8 = persist.tile([1, ST * C], F32)
                g_row = g_row8[:, 0:C]
                nc.vector.reciprocal(g_row, fr_row)
                if KL_DT == FP16:
                    # keep v = q*g and a = q*v inside fp16's normal range;
                    # kl_tok = L/A - ln A is exactly invariant to this scale
                    nc.vector.tensor_scalar_mul(g_row, g_row, 4096.0)
                for g in range(1, ST):
                    nc.vector.tensor_copy(g_row8[:, g * C:(g + 1) * C],
                                          g_row)
                grep_p = ps_bc.tile([128, ST * C], F32)
                nc.tensor.matmul(grep_p, ones_r1, g_row8, start=True,
                                 stop=True)

                # ---------- sweep B (outputs), then sweep C (KL) ---------
                # All of B is emitted before any C op so the per-engine
                # static order lets B proceed while the collective that
                # gates C is still in flight.
                with (
                    tc.tile_pool(name="ps_wt", bufs=2, space="PSUM") as ps_wt,
                    tc.tile_pool(name="ps_xc", bufs=2, space="PSUM") as ps_xc,
                ):
                    def emit_B(st):
                        row0 = st * ST * GRP
                        xs = xs_arr[:, row0:row0 + ST * GRP]
                        wd_sl = wd_arr[:, st * ST * C:(st + 1) * ST * C]

                        wT_p = ps_wt.tile([C, ST * 128], BF16, tag="wT_p")
                        for g in range(ST):
                            nc.tensor.transpose(
                                wT_p[:, g * 128:(g + 1) * 128],
                                wd_sl[:, g * C:(g + 1) * C], identb)
                        wT_sb = work.tile([C, ST * 128], BF16, tag="wT")
                        nc.scalar.copy(wT_sb, wT_p)

                        for h in range(2):
                            xc_p = ps_xc.tile([128, 512], F32, tag="xc_p")
                            for j in range(4):
                                g = h * 4 + j
                                nc.tensor.matmul(
                                    xc_p[:, j * 128:(j + 1) * 128],
                                    wT_sb[:, g * 128:(g + 1) * 128],
                                    dictbf_sb, start=True, stop=True)
                            res_sb = work.tile([128, 512], F32, tag="res")
                            nc.vector.tensor_sub(
                                res_sb, xs[:, h * 512:(h + 1) * 512], xc_p)
                            xc_sb = work.tile([128, 512], F32, tag="xc")
                            nc.scalar.copy(xc_sb, xc_p)
                            lo = row0 + h * 4 * GRP
                            nc.sync.dma_start(
                                xc_d.ap()[lo:lo + 512, :].rearrange(
                                    "(g p) d -> p g d", p=128),
                                xc_sb.rearrange("p (g d) -> p g d", d=128))
                            nc.sync.dma_start(
                                xr_d.ap()[lo:lo + 512, :].rearrange(
                                    "(g p) d -> p g d", p=128),
                                res_sb.rearrange("p (g d) -> p g d", d=128))

                    def emit_C(st):
                        q_sl = q_arr[:, st * ST * C:(st + 1) * ST * C]
                        v_sb = work.tile([128, ST * C], KL_DT, tag="v")
                        nc.vector.tensor_tensor(v_sb, q_sl, grep_p, OP.mult)
                        a_sb = work.tile([128, ST * C], KL_DT, tag="a")
                        nc.vector.tensor_tensor(a_sb, q_sl, v_sb, OP.mult)
                        u_sb = work.tile([128, ST * C], KL_DT, tag="u")
                        nc.scalar.activation(u_sb, v_sb, AF.Ln)
                        t_sb = work.tile([128, ST * C], KL_DT, tag="t")
                        nc.vector.tensor_tensor(t_sb, a_sb, u_sb, OP.mult)
                        nc.vector.tensor_reduce(
                            A_arr[:, st * ST:(st + 1) * ST],
                            a_sb.rearrange("p (g c) -> p g c", c=C),
                            AX.X, OP.add)
                        nc.vector.tensor_reduce(
                            L_arr[:, st * ST:(st + 1) * ST],
                            t_sb.rearrange("p (g c) -> p g c", c=C),
                            AX.X, OP.add)

                    # B first; C offset so the collective completes before
                    # the DVE's static program order reaches C(0)
                    for st in range(NST):
                        emit_B(st)
                        if st >= C_OFFSET:
                            emit_C(st - C_OFFSET)
                    for st in range(NST - C_OFFSET, NST):
                        emit_C(st)

                    # ---- KL tail ----
                    rA = persist.tile([128, NGRP], F32)
                    nc.vector.reciprocal(rA, A_arr)
                    lnA = persist.tile([128, NGRP], F32)
                    nc.scalar.activation(lnA, A_arr, AF.Ln)
                    kt = persist.tile([128, NGRP], F32)
                    nc.vector.tensor_tensor(kt, L_arr, rA, OP.mult)
                    nc.vector.tensor_sub(kt, kt, lnA)
                    klcol = persist.tile([128, 1], F32)
                    nc.vector.tensor_reduce(klcol, kt, AX.X, OP.add)
                    kl_p = ps_bc.tile([1, 1], F32)
                    nc.tensor.matmul(kl_p, klcol, ones_col, start=True,
                                     stop=True)
                    kl_sb = persist.tile([1, 1], F32)
                    nc.scalar.mul(kl_sb, kl_p, 0.5 / M_TOTAL)
                    nc.sync.dma_start(kl_d.ap(), kl_sb)

    nc.compile()
    return nc


def kernel(x: np.ndarray, dictionary: np.ndarray):
    if "nc" not in _CACHED:
        _CACHED["nc"] = _build()
    nc = _CACHED["nc"]

    x_flat = np.ascontiguousarray(x.reshape(M_TOTAL, D).astype(np.float32))
    dT = np.ascontiguousarray(dictionary.T.astype(np.float32))
    dbf = np.ascontiguousarray(dictionary.astype(ml_dtypes.bfloat16))

    in_maps = [{
        "x": x_flat[c * TOK:(c + 1) * TOK],
        "dictT": dT,
        "dictbf": dbf,
    } for c in range(NCORES)]

    res = bass_utils.run_bass_kernel_spmd(nc, in_maps,
                                          core_ids=list(range(NCORES)))
    outs = res.results

    xc = np.concatenate([outs[c]["xc"] for c in range(NCORES)], axis=0)
    xr = np.concatenate([outs[c]["xr"] for c in range(NCORES)], axis=0)
    kl_scaled = np.sum([outs[c]["kl"][0, 0] for c in range(NCORES)],
                       dtype=np.float32)
    ortho = outs[0]["ortho"][0, 0]
    aux = np.float32(kl_scaled + np.float32(0.1) * ortho)

    return (xc.reshape(B, N, D), xr.reshape(B, N, D), aux)
